# revision 45
# baseline (speedup 1.0000x reference)
"""RNN-T decoder (embedding + 2-layer LSTM + joint network) on 8 Trainium2 cores.

v2 strategy (fp8 DoubleRow recurrence):
  - LSTM runs replicated on all 8 cores. The recurrent matmul streams W_hh as
    fp8(e4m3, x64) in DoubleRow perf mode (256 contraction rows per pass, 0.5
    cycles/output-row): 4x less PE time than the fp32r baseline. h is
    quantized to fp8 (x64) each step; the batched x-projection gx is
    quantized to fp8 (x32) and injected into PSUM via a DoubleRow matmul
    against a constant "double identity" whose entries are 128, so all PSUM
    contributions carry the same 4096x scale. Activations then apply
    tanh(ps/8192) (== sigmoid pre-halving) or tanh(ps/4096) for the g gate.
  - The elementwise c/h chain runs in a TRANSPOSED [128, 32] layout (gate
    values are transposed by the PE right after activation), which cuts
    DVE/ACT cost ~4x vs the [4, 512] layout and directly produces h^T tiles
    for the next step's matmul (fp8) and for xproj/joint (bf16).
  - Joint network is sharded over T (16 cols/core); zt/W_dec/W_out in bf16.

kernel(**inputs) takes the full unsharded inputs (as in reference.setup_inputs)
and returns the full (B, T, U, ODIM) float32 output.
"""
import sys
import numpy as np

sys.path.insert(0, "/opt/trn_rl_repo")

import concourse.bass as bass
import concourse.bacc as bacc
import concourse.mybir as mybir
import concourse.tile as tile
from contextlib import ExitStack

F32 = mybir.dt.float32
F32R = mybir.dt.float32r
BF16 = mybir.dt.bfloat16
F8 = mybir.dt.float8e4
I32 = mybir.dt.int32
AF = mybir.ActivationFunctionType
OP = mybir.AluOpType
DR = mybir.MatmulPerfMode.DoubleRow

B, T, U = 4, 128, 64
NCORES = 8
TC = T // NCORES          # 16 T-columns per core
E, D, G = 512, 1024, 4096  # embed, dunits, 4*dunits
J, O = 512, 2048           # joint dim, odim
UB = U * B                 # 256, u-major token index (col = 4u+b)
BT = B * TC                # 64 encoder rows per core
KD = D // 128              # 8 contraction chunks of hidden dim
DC = D // 256              # 4 double-chunks for DoubleRow

_CACHE = {}
JZT, JOSB, JPJ = 4, 4, 4
PBIG = 6
PTT = 1


def _mm_r(nc, out, lhsT, rhs, **kw):
    """matmul with both operands viewed as float32r (full-rate fp32 storage)."""
    nc.tensor.matmul(out, lhsT=lhsT.bitcast(F32R), rhs=rhs.bitcast(F32R), **kw)


def _emit_xproj_q(nc, pools, rhs_of_ec, w_of_ec, nchunks, bih, gx8,
                  lhsT_bitcast=None):
    """gx8[2*nb+cp][:, i*256:(i+1)*256] = fp8(16 * ((W @ x^T) + bias)).

    rhs_of_ec: ec -> [128, 256] rhs tile (x^T chunk, K on partitions)
    w_of_ec:   ec -> [128, G] tile of W.T rows for that chunk (lhsT source)
    """
    pbig, bpool = pools["pbig"], pools["bias"]
    ones_r = pools["ones"]
    # one host-summed bias vector, loaded in a single DMA upfront (tiny DMAs
    # inside the loop would queue behind the weight streams and stall the
    # bias-inject matmuls)
    ba_all = bpool.tile([1, G], F32R, tag="ba_all", name="ba_all")
    nc.sync.dma_start(ba_all[:1, :], bih[None, :].bitcast(F32R))
    for gm in range(32):
        nb, c = gm // 4, gm % 4
        gs = slice(128 * c, 128 * c + 128)
        ps = pbig.tile([128, 256], F32, tag="pbig", name="pbig")
        for ec in range(nchunks):
            w = w_of_ec(ec)[:, 128 * gm: 128 * (gm + 1)]
            r = rhs_of_ec(ec)
            if lhsT_bitcast is None:
                _mm_r(nc, ps[:], lhsT=w, rhs=r,
                      start=(ec == 0), stop=False)
            else:
                nc.tensor.matmul(ps[:], lhsT=w, rhs=r,
                                 start=(ec == 0), stop=False)
        _mm_r(nc, ps[:], lhsT=ba_all[:1, 512 * nb + 128 * c:
                                     512 * nb + 128 * c + 128],
              rhs=ones_r[:1, :256], start=False, stop=True)
        # quantize to fp8 with x16 scale into the paired layout
        nc.vector.tensor_scalar(
            gx8[2 * nb + c // 2][:, (c % 2) * 256:(c % 2) * 256 + 256],
            ps[:], 32.0, None, OP.mult)


def _emit_lstm_fp8(nc, pools, whh8, gx8, hT16, cT, h8ab, steps, heat=0):
    """One LSTM layer in fp8 DoubleRow, `steps` sequential steps.

    whh8: 4 SBUF tiles [128, 2*G] fp8 (64x W_hh.T), dc-major double-chunks
    gx8: 16 SBUF tiles [128, 512] fp8 (16x (W_ih x + b)), paired layout
    hT16: [128, 32*(U+1)] bf16; col block u+1 <- h[u]^T, cols (i,dc,b)-major
    cT: [128, 32] f32 persistent (caller memsets)
    h8ab: two [128, 32] fp8 tiles, ping-pong (caller memsets h8ab[0])
    """
    pg8, ptT = pools["pgates"], pools["ptT"]
    tpool = pools["tsb"]
    ident8 = pools["ident8"]
    identb = pools["identb"]
    chain = pools["chain"]
    tc_t = pools["tc_t"]
    # processing order: i, g, f, o  (gate index 0,2,1,3) so the post-o tail
    # is only so/h8/h16
    ORDER = (0, 2, 1, 3)
    for u in range(steps):
        h8prev = h8ab[u % 2]
        h8cur = h8ab[(u + 1) % 2]
        lhs_h = h8prev[:].rearrange("p (i dc b) -> p dc i b", i=2, dc=DC, b=4)
        tT = ptT.tile([128, 128], BF16, tag="tT", name="tT")
        tsb = {}
        # --- all matmuls first (PE queue order) ---
        pgs = {}
        inj_first = pools.get("inj_first", False)
        for gt in ORDER:
            pg = pg8.tile([4, 1024], F32, tag="pg", name="pg")
            pgs[gt] = pg
            for nh in range(2):
                nb = 2 * gt + nh
                out = pg[:, 512 * nh: 512 * nh + 512]
                if inj_first:
                    for cp in range(2):
                        lg = gx8[2 * nb + cp][:].rearrange(
                            "p (i ub) -> p i ub", i=2)[:, :, 4 * u: 4 * u + 4]
                        nc.tensor.matmul(
                            pg[:, 512 * nh + 256 * cp: 512 * nh + 256 * cp + 256],
                            lhsT=lg, rhs=ident8[:].rearrange(
                                "p (i n) -> p i n", i=2),
                            start=(cp == 0), stop=False, perf_mode=DR)
                for dc in range(DC):
                    rhs = whh8[dc][:].rearrange("p (i n) -> p i n", i=2)[
                        :, :, 512 * nb: 512 * nb + 512]
                    nc.tensor.matmul(out, lhsT=lhs_h[:, dc], rhs=rhs,
                                     start=(dc == 0 and not inj_first),
                                     stop=(inj_first and dc == DC - 1),
                                     perf_mode=DR)
                if inj_first:
                    continue
                if "no_inj" in pools["ablate"]:
                    nc.tensor.matmul(
                        pg[:, 512 * nh: 512 * nh + 4], lhsT=lhs_h[:, 0],
                        rhs=whh8[0][:].rearrange("p (i n) -> p i n", i=2)[:, :, :4],
                        start=False, stop=True, perf_mode=DR)
                else:
                    for cp in range(2):
                        lg = gx8[2 * nb + cp][:].rearrange(
                            "p (i ub) -> p i ub", i=2)[:, :, 4 * u: 4 * u + 4]
                        nc.tensor.matmul(
                            pg[:, 512 * nh + 256 * cp: 512 * nh + 256 * cp + 256],
                            lhsT=lg, rhs=ident8[:].rearrange(
                                "p (i n) -> p i n", i=2),
                            start=False, stop=(cp == 1), perf_mode=DR)
        # --- activations + transposes + chain, in processing order ---
        dve_gates = pools.get("dve_gates") or ()
        split_first = pools.get("split_first", 0)
        for oi, gt in enumerate(ORDER):
            scale = (1.0 / 4096.0) if gt == 2 else (1.0 / 8192.0)
            t_sb = tpool.tile([4, 1024], BF16, tag="tsb", name="tsb")
            tsb[gt] = t_sb
            if "tiny_act" in pools["ablate"]:
                nc.scalar.activation(t_sb[:, :4], pgs[gt][:, :4], AF.Tanh,
                                     scale=scale)
                nc.gpsimd.memset(t_sb[:, 4:].bitcast(mybir.dt.uint16), 0)
            elif gt in dve_gates:
                # drain PSUM on the DVE (raw, pre-scaled); tanh after transpose
                nc.vector.tensor_scalar(t_sb[:], pgs[gt][:], scale, None,
                                        OP.mult)
            elif oi < split_first:
                # early gates: act per 512-half (each half is its own PSUM
                # accumulation group) so ACT starts half a block sooner
                for nh in range(2):
                    nc.scalar.activation(t_sb[:, 512 * nh:512 * nh + 512],
                                         pgs[gt][:, 512 * nh:512 * nh + 512],
                                         AF.Tanh, scale=scale)
            else:
                nc.scalar.activation(t_sb[:], pgs[gt][:], AF.Tanh, scale=scale)
            ntr = 1 if "no_tr" in pools["ablate"] else 8
            for k in range(ntr):
                ck = 16 * (k % 2) + 4 * (k // 2)
                nc.tensor.transpose(
                    tT[:, 32 * gt + ck: 32 * gt + ck + 4],
                    in_=t_sb[:, 128 * k: 128 * k + 128],
                    identity=identb[:4, :4])
            if ntr == 1:
                nc.gpsimd.memset(tT[:, 32 * gt + 4: 32 * gt + 32].bitcast(mybir.dt.uint16), 0)
            ts = tT[:, 32 * gt: 32 * gt + 32]
            if gt in dve_gates:
                ta = chain.tile([128, 32], BF16, tag=f"ta{gt}", name=f"ta{gt}")
                nc.scalar.activation(ta[:], ts, AF.Tanh)
                ts = ta[:]
            if gt == 0:      # i
                si = chain.tile([128, 32], F32, tag="si", name="si")
                pools["si"] = si
                nc.vector.tensor_scalar(si[:], ts, 0.5, 0.5, OP.mult, OP.add)
            elif gt == 2:    # g
                ig = chain.tile([128, 32], F32, tag="ig", name="ig")
                pools["ig"] = ig
                nc.vector.tensor_tensor(ig[:], in0=pools["si"][:], in1=ts,
                                        op=OP.mult)
            elif gt == 1:    # f
                sf = chain.tile([128, 32], F32, tag="sf", name="sf")
                nc.vector.tensor_scalar(sf[:], ts, 0.5, 0.5, OP.mult, OP.add)
                nc.vector.tensor_tensor(sf[:], in0=sf[:], in1=cT[:], op=OP.mult)
                nc.vector.tensor_tensor(cT[:], in0=sf[:], in1=pools["ig"][:],
                                        op=OP.add)
            else:            # o
                so = chain.tile([128, 32], F32, tag="so", name="so")
                pools["so"] = so
                nc.vector.tensor_scalar(so[:], ts, 32.0, 32.0, OP.mult, OP.add)
        # tanh(c') queues after the o activation so act_o isn't delayed
        nc.scalar.activation(tc_t[:], cT[:], AF.Tanh)
        nc.vector.tensor_tensor(h8cur[:], in0=pools["so"][:], in1=tc_t[:],
                                op=OP.mult)
        nc.vector.tensor_scalar(
            hT16[:, 32 * (u + 1): 32 * (u + 2)], h8cur[:],
            1.0 / 64.0, None, OP.mult)
        # optional PE heater: keep the tensor engine clocked during the tail
        for _ in range(heat):
            hp = pools["pheat"].tile([4, 512], F32, tag="ph", name="ph")
            nc.tensor.matmul(hp[:], lhsT=lhs_h[:, 0],
                             rhs=whh8[0][:].rearrange(
                                 "p (i n) -> p i n", i=2)[:, :, :512],
                             start=True, stop=True, perf_mode=DR)


def build_nc(steps=U, layers=2, joint=True, heat=2, pg_bufs=3,
             dve_gates=(), prefetch=1, ablate=(), inj_first=1, split_first=2):
    nc = bacc.Bacc("TRN2", target_bir_lowering=False, debug=False)

    hs = nc.dram_tensor("hs", [BT, E], F32, kind="ExternalInput")
    ys_idx = nc.dram_tensor("ys_idx", [UB], I32, kind="ExternalInput")
    embed = nc.dram_tensor("embed", [O, E], F32, kind="ExternalInput")
    wih0T = nc.dram_tensor("wih0T", [E, G], BF16, kind="ExternalInput")
    wih1T16 = nc.dram_tensor("wih1T16", [D, G], BF16, kind="ExternalInput")
    whh8_0 = nc.dram_tensor("whh8_0", [4 * 128, 2 * G], F8, kind="ExternalInput")
    whh8_1 = nc.dram_tensor("whh8_1", [4 * 128, 2 * G], F8, kind="ExternalInput")
    bih0 = nc.dram_tensor("bih0", [G], F32, kind="ExternalInput")
    bih1 = nc.dram_tensor("bih1", [G], F32, kind="ExternalInput")
    wencT = nc.dram_tensor("wencT", [E, J], F32R, kind="ExternalInput")
    benc = nc.dram_tensor("benc", [J], F32R, kind="ExternalInput")
    wdecT16 = nc.dram_tensor("wdecT16", [D, J], BF16, kind="ExternalInput")
    woutT16 = nc.dram_tensor("woutT16", [J, O], BF16, kind="ExternalInput")
    bout_bc = nc.dram_tensor("bout_bc", [128, O], BF16, kind="ExternalInput")
    ones_d = nc.dram_tensor("ones_d", [256], F32R, kind="ExternalInput")
    ident_f = nc.dram_tensor("ident_f", [128, 128], F32, kind="ExternalInput")
    ident_b = nc.dram_tensor("ident_b", [128, 128], BF16, kind="ExternalInput")
    ident_8 = nc.dram_tensor("ident_8", [128, 512], F8, kind="ExternalInput")
    out = nc.dram_tensor("out", [BT * U, O], F32, kind="ExternalOutput")

    with tile.TileContext(nc) as tc, ExitStack() as es:
        cpool = es.enter_context(tc.tile_pool(name="const", bufs=1))
        ppool = es.enter_context(tc.tile_pool(name="persist", bufs=1))

        ident = cpool.tile([128, 128], F32, tag="ident", name="ident")
        nc.sync.dma_start(ident[:], ident_f[:])
        ones_r = cpool.tile([1, 256], F32R, tag="ones", name="ones")
        nc.sync.dma_start(ones_r[:1, :], ones_d[None, :])
        identb = cpool.tile([128, 128], BF16, tag="identb", name="identb")
        nc.sync.dma_start(identb[:], ident_b[:])
        ident8 = cpool.tile([128, 512], F8, tag="ident8", name="ident8")
        nc.sync.dma_start(ident8[:], ident_8[:])

        gx8 = [ppool.tile([128, 512], F8, tag=f"gx8_{g}", name=f"gx8_{g}")
               for g in range(16)]
        hT16_0 = ppool.tile([128, 32 * (U + 1)], BF16, tag="hT16_0", name="hT16_0")
        hT16_1 = ppool.tile([128, 32 * (U + 1)], BF16, tag="hT16_1", name="hT16_1")
        cT = ppool.tile([128, 32], F32, tag="cT", name="cT")
        h8a = ppool.tile([128, 32], F8, tag="h8a", name="h8a")
        h8b = ppool.tile([128, 32], F8, tag="h8b", name="h8b")
        tc_t = ppool.tile([128, 32], BF16, tag="tc_t", name="tc_t")
        whh0 = [ppool.tile([128, 2 * G], F8, tag=f"whh0_{d}", name=f"whh0_{d}")
                for d in range(DC)]
        whh1 = [ppool.tile([128, 2 * G], F8, tag=f"whh1_{d}", name=f"whh1_{d}")
                for d in range(DC)]

        pools = {"ones": ones_r, "identb": identb, "ident8": ident8,
                 "tc_t": tc_t, "dve_gates": tuple(dve_gates),
                 "ablate": tuple(ablate), "inj_first": bool(inj_first),
                 "split_first": split_first}

        wenc = [ppool.tile([128, J], F32R, tag=f"wenc{ec}", name=f"wenc{ec}")
                for ec in range(4)]
        benc_sb = ppool.tile([1, J], F32R, tag="benc", name="benc")
        hs_sb = ppool.tile([BT, E], F32, tag="hs_sb", name="hs_sb")
        hsT = [ppool.tile([128, BT], F32R, tag=f"hsT{ec}", name=f"hsT{ec}")
               for ec in range(4)]
        encp = [ppool.tile([128, BT], F32, tag=f"encp{jt}", name=f"encp{jt}")
                for jt in range(4)]

        # ---- Phase 1+2: embedding gather -> eys^T, layer-0 x-projection ----
        with tc.tile_pool(name="ph2", bufs=1) as p2, \
             tc.tile_pool(name="bias2", bufs=2) as bpool2, \
             tc.tile_pool(name="pbig", bufs=PBIG, space="PSUM") as pbig, \
             tc.tile_pool(name="pT", bufs=2, space="PSUM") as pT:
            pools["pbig"] = pbig
            pools["bias"] = bpool2
            eysT = [p2.tile([128, 256], BF16, tag=f"eysT{ec}", name=f"eysT{ec}")
                    for ec in range(4)]
            idxs = []
            for t in range(2):
                idx = p2.tile([128, 1], I32, tag=f"idx{t}", name=f"idx{t}")
                nc.sync.dma_start(idx[:, :1], ys_idx[128 * t:128 * (t + 1), None])
                idxs.append(idx)
            wih0 = [p2.tile([128, G], BF16, tag=f"wih0_{ec}", name=f"wih0_{ec}")
                    for ec in range(4)]
            for ec in range(4):
                nc.sync.dma_start(wih0[ec][:], wih0T[128 * ec:128 * (ec + 1), :])
            # recurrent weights (both layers) early: overlaps with xproj
            for d in range(DC):
                nc.sync.dma_start(whh0[d][:], whh8_0[128 * d:128 * (d + 1), :])
            for t in range(2):
                idx = idxs[t]
                ey = p2.tile([128, E], F32, tag=f"ey{t}", name=f"ey{t}")
                nc.gpsimd.indirect_dma_start(
                    out=ey[:], out_offset=None, in_=embed[:],
                    in_offset=bass.IndirectOffsetOnAxis(ap=idx[:, :1], axis=0))
                for ec in range(4):
                    tp = pT.tile([128, 128], F32, tag="pT", name="pT")
                    nc.tensor.transpose(tp[:], in_=ey[:, 128 * ec:128 * (ec + 1)],
                                        identity=ident[:])
                    nc.vector.tensor_copy(eysT[ec][:, 128 * t:128 * (t + 1)], tp[:])
            _emit_xproj_q(nc, pools, lambda ec: eysT[ec][:],
                          lambda ec: wih0[ec], 4, bih0, gx8,
                          lhsT_bitcast=True)

            for d in range(DC):
                nc.sync.dma_start(whh1[d][:], whh8_1[128 * d:128 * (d + 1), :])
            # encoder-side joint work: hs^T and enc_p^T (PE has slack here)
            for ec in range(4):
                nc.sync.dma_start(wenc[ec][:], wencT[128 * ec:128 * (ec + 1), :])
            nc.sync.dma_start(hs_sb[:], hs[:])
            nc.sync.dma_start(benc_sb[:1, :], benc[None, :])
            for ec in range(4):
                tp = pT.tile([128, 128], F32, tag="pT", name="pT")
                nc.tensor.transpose(tp[:, :BT],
                                    in_=hs_sb[:, 128 * ec:128 * (ec + 1)],
                                    identity=ident[:BT, :BT])
                nc.vector.tensor_copy(hsT[ec][:], tp[:, :BT])
            for jt in range(4):
                tpp = pT.tile([128, 128], F32, tag="pT", name="pT")
                pse = tpp[:, :BT]
                for ec in range(4):
                    _mm_r(nc, pse, lhsT=wenc[ec][:, 128 * jt:128 * (jt + 1)],
                          rhs=hsT[ec][:], start=(ec == 0), stop=False)
                _mm_r(nc, pse, lhsT=benc_sb[:1, 128 * jt:128 * (jt + 1)],
                      rhs=ones_r[:1, :BT], start=False, stop=True)
                nc.vector.tensor_copy(encp[jt][:], pse)

        # ---- Phase 3: layer-0 recurrence ----
        wih1p = es.enter_context(tc.tile_pool(name="wih1p", bufs=1))
        wih1 = [wih1p.tile([128, G], BF16, tag=f"wih1_{k}", name=f"wih1_{k}")
                for k in range(KD)]
        if prefetch:
            for k in range(KD):
                nc.sync.dma_start(wih1[k][:], wih1T16[128 * k:128 * (k + 1), :])
        if steps < U:  # debug builds: phases 4/6 read all U step blocks
            nc.gpsimd.memset(hT16_0[:], 0.0)
            nc.gpsimd.memset(hT16_1[:], 0.0)
        nc.gpsimd.memset(cT[:], 0.0)
        nc.gpsimd.memset(h8a[:].bitcast(mybir.dt.uint8), 0)
        with tc.tile_pool(name="pgA", bufs=pg_bufs, space="PSUM") as pgates, \
             tc.tile_pool(name="ptTA", bufs=PTT, space="PSUM") as ptT, \
             tc.tile_pool(name="pheatA", bufs=1, space="PSUM") as pheat, \
             tc.tile_pool(name="tsbA", bufs=3) as tsb, \
             tc.tile_pool(name="chainA", bufs=2) as chain:
            pools.update(pgates=pgates, ptT=ptT, tsb=tsb, chain=chain,
                         pheat=pheat)
            _emit_lstm_fp8(nc, pools, whh0, gx8, hT16_0, cT, (h8a, h8b),
                           steps, heat=heat)

        # ---- Phase 4: layer-1 x-projection (streams W_ih1 bf16) ----
        with tc.tile_pool(name="bias4", bufs=2) as bpool4, \
             tc.tile_pool(name="pbig2", bufs=4, space="PSUM") as pbig2:
            pools["pbig"] = pbig2
            pools["bias"] = bpool4
            if not prefetch:
                for k in range(KD):
                    nc.sync.dma_start(wih1[k][:],
                                      wih1T16[128 * k:128 * (k + 1), :])
            rhs_of = lambda ec: hT16_0[:].rearrange(
                "p (u c b) -> p c u b", u=U + 1, c=8, b=4)[
                :, 4 * (ec % 2) + ec // 2, 1:U + 1]
            _emit_xproj_q(nc, pools, rhs_of, lambda ec: wih1[ec], KD,
                          bih1, gx8, lhsT_bitcast=True)

        # ---- Phase 5: layer-1 recurrence ----
        if joint:
            jp = es.enter_context(tc.tile_pool(name="joint", bufs=1))
            wdec = [jp.tile([128, J], BF16, tag=f"wdec{k}", name=f"wdec{k}")
                    for k in range(KD)]
            wout = [jp.tile([128, O], BF16, tag=f"wout{jt}", name=f"wout{jt}")
                    for jt in range(4)]
            bout_sb = jp.tile([128, O], BF16, tag="bout", name="bout")
            if prefetch:
                for k in range(KD):
                    nc.sync.dma_start(wdec[k][:],
                                      wdecT16[128 * k:128 * (k + 1), :])
                nc.sync.dma_start(bout_sb[:], bout_bc[:])
                for jt in range(4):
                    nc.sync.dma_start(wout[jt][:],
                                      woutT16[128 * jt:128 * (jt + 1), :])
        nc.gpsimd.memset(cT[:], 0.0)
        nc.gpsimd.memset(h8a[:].bitcast(mybir.dt.uint8), 0)
        if layers >= 2:
            with tc.tile_pool(name="pgB", bufs=pg_bufs, space="PSUM") as pgates2, \
                 tc.tile_pool(name="ptTB", bufs=PTT, space="PSUM") as ptT2, \
                 tc.tile_pool(name="pheatB", bufs=1, space="PSUM") as pheat2, \
                 tc.tile_pool(name="tsbB", bufs=3) as tsb2, \
                 tc.tile_pool(name="chainB", bufs=2) as chain2:
                pools.update(pgates=pgates2, ptT=ptT2, tsb=tsb2, chain=chain2,
                             pheat=pheat2)
                _emit_lstm_fp8(nc, pools, whh1, gx8, hT16_1, cT, (h8a, h8b),
                               steps, heat=heat)
        else:
            nc.vector.tensor_copy(hT16_1[:, 32:32 * (steps + 1)],
                                  hT16_0[:, 32:32 * (steps + 1)])

        # ---- Phase 6: joint network on this core's T-slice ----
        if joint:
            with tc.tile_pool(name="jointl", bufs=1) as jp2, \
                 tc.tile_pool(name="zt", bufs=JZT) as ztp, \
                 tc.tile_pool(name="osb", bufs=JOSB) as osbp, \
                 tc.tile_pool(name="pj", bufs=JPJ, space="PSUM") as pj:
                decp = [jp2.tile([128, 256], F32, tag=f"decp{jt}", name=f"decp{jt}")
                        for jt in range(4)]
                if not prefetch:
                    for k in range(KD):
                        nc.sync.dma_start(wdec[k][:],
                                          wdecT16[128 * k:128 * (k + 1), :])
                    nc.sync.dma_start(bout_sb[:], bout_bc[:])
                    for jt in range(4):
                        nc.sync.dma_start(wout[jt][:],
                                          woutT16[128 * jt:128 * (jt + 1), :])

                # dec_p^T[jt], columns reordered (b, u)
                for jt in range(4):
                    ps = pj.tile([128, 256], F32, tag="pj", name="pj")
                    for k in range(KD):
                        rhs = hT16_1[:].rearrange(
                            "p (u c b) -> p c b u", u=U + 1, c=8, b=4)[
                            :, 4 * (k % 2) + k // 2, :, 1:U + 1]
                        nc.tensor.matmul(ps[:], lhsT=wdec[k][:, 128 * jt:128 * (jt + 1)],
                                         rhs=rhs, start=(k == 0), stop=(k == KD - 1))
                    nc.vector.tensor_copy(decp[jt][:], ps[:])
                # z^T tiles + output matmul, one M-tile (=2 encoder rows) at a time
                for m in range(BT * U // 128):
                    zt = [ztp.tile([128, 128], BF16, tag=f"zt{jt}", name=f"zt{jt}")
                          for jt in range(4)]
                    for jt in range(4):
                        for half in range(2):
                            bt = 2 * m + half
                            b = bt // TC
                            nc.scalar.activation(
                                zt[jt][:, half * 64:(half + 1) * 64],
                                decp[jt][:, b * 64:(b + 1) * 64],
                                AF.Tanh, bias=encp[jt][:, bt:bt + 1])
                    for ob in range(4):
                        obs = slice(ob * 512, (ob + 1) * 512)
                        ps = pj.tile([128, 512], F32, tag="pj", name="pj")
                        for jt in range(4):
                            nc.tensor.matmul(ps[:], lhsT=zt[jt][:],
                                             rhs=wout[jt][:, obs],
                                             start=(jt == 0), stop=(jt == 3))
                        o_sb = osbp.tile([128, 512], F32, tag="osb", name="osb")
                        nc.vector.tensor_tensor(o_sb[:], in0=ps[:],
                                                in1=bout_sb[:, obs], op=OP.add)
                        nc.sync.dma_start(out[128 * m:128 * (m + 1), obs], o_sb[:])
        else:
            zsrc = ppool.tile([128, 512], F32, tag="zsrc", name="zsrc")
            nc.gpsimd.memset(zsrc[:], 0.0)
            for m0 in range(BT * U // 128):
                for ob in range(4):
                    nc.sync.dma_start(out[128 * m0:128 * (m0 + 1),
                                          ob * 512:(ob + 1) * 512], zsrc[:])

    nc.compile()
    return nc


def _prep_inputs(hs_pad, ys_in_pad, embed, W_ih0, W_hh0, b_ih0, b_hh0,
                 W_ih1, W_hh1, b_ih1, b_hh1, W_enc, b_enc, W_dec, W_out, b_out):
    import ml_dtypes
    E4 = ml_dtypes.float8_e4m3
    BF = ml_dtypes.bfloat16
    f = np.float32
    tr = lambda a: np.ascontiguousarray(np.asarray(a).T, dtype=f)

    def whh8(W):
        WT64 = (np.asarray(W, f).T * 64.0).astype(E4)          # [D, G]
        return np.ascontiguousarray(
            WT64.reshape(DC, 2, 128, G).transpose(0, 2, 1, 3).reshape(4 * 128, 2 * G))

    id8 = np.zeros((128, 512), E4)
    for p in range(128):
        id8[p, p] = 128.0
        id8[p, 256 + 128 + p] = 128.0

    common = {
        "ys_idx": np.ascontiguousarray(np.asarray(ys_in_pad).T.reshape(-1),
                                       dtype=np.int32),
        "embed": np.ascontiguousarray(embed, dtype=f),
        "wih0T": np.ascontiguousarray(np.asarray(W_ih0, f).T.astype(BF)),
        "wih1T16": np.ascontiguousarray(np.asarray(W_ih1, f).T.astype(BF)),
        "whh8_0": whh8(W_hh0), "whh8_1": whh8(W_hh1),
        "bih0": np.asarray(b_ih0, f) + np.asarray(b_hh0, f),
        "bih1": np.asarray(b_ih1, f) + np.asarray(b_hh1, f),
        "wencT": tr(W_enc),
        "wdecT16": np.ascontiguousarray(np.asarray(W_dec, f).T.astype(BF)),
        "woutT16": np.ascontiguousarray(np.asarray(W_out, f).T.astype(BF)),
        "benc": np.asarray(b_enc, f),
        "bout_bc": np.ascontiguousarray(
            np.broadcast_to(np.asarray(b_out, f).astype(BF)[None, :], (128, O))),
        "ones_d": np.ones(256, f),
        "ident_f": np.eye(128, dtype=f),
        "ident_b": np.eye(128).astype(BF),
        "ident_8": id8,
    }
    hs_np = np.asarray(hs_pad, f)
    in_maps = []
    for c in range(NCORES):
        m = dict(common)
        m["hs"] = np.ascontiguousarray(
            hs_np[:, c * TC:(c + 1) * TC, :].reshape(BT, E))
        in_maps.append(m)
    return in_maps



def _get_runner():
    """Build (once) a reusable jitted SPMD callable.

    Weights are replicated across the 8 cores (in_specs=P()); only hs and the
    output are sharded over the leading axis. This avoids the 8x concat +
    retrace of run_bass_kernel_spmd on every call.
    """
    if "runner" in _CACHE:
        return _CACHE["runner"]
    import jax
    from jax.sharding import Mesh, PartitionSpec as P
    from jax.experimental.shard_map import shard_map
    from concourse import bass2jax
    import concourse.mybir as mybir_

    nc = _CACHE.get("nc")
    if nc is None:
        nc = _CACHE["nc"] = build_nc()
    bass2jax.install_neuronx_cc_hook()

    pname = nc.partition_id_tensor.name if nc.partition_id_tensor else None
    in_names, out_names, out_avals = [], [], []
    for alloc in nc.m.functions[0].allocations:
        if not isinstance(alloc, mybir_.MemoryLocationSet):
            continue
        name = alloc.memorylocations[0].name
        if alloc.kind == "ExternalInput":
            if name != pname:
                in_names.append(name)
        elif alloc.kind == "ExternalOutput":
            out_names.append(name)
            shape = tuple(alloc.tensor_shape)
            out_avals.append(jax.core.ShapedArray(shape, mybir_.dt.np(alloc.dtype)))
    n_params = len(in_names)
    all_names = in_names + out_names
    if pname is not None:
        all_names = all_names + [pname]

    def _body(*args):
        operands = list(args)
        if pname is not None:
            operands.append(bass2jax.partition_id_tensor())
        outs = bass2jax._bass_exec_p.bind(
            *operands,
            out_avals=tuple(out_avals),
            in_names=tuple(all_names),
            out_names=tuple(out_names),
            lowering_input_output_aliases=(),
            sim_require_finite=True,
            sim_require_nnan=True,
            nc=nc,
        )
        return tuple(outs)

    devices = jax.devices()[:NCORES]
    mesh = Mesh(np.asarray(devices), ("core",))
    in_specs = tuple(P("core") if n == "hs" else P() for n in in_names)
    in_specs = in_specs + (P("core"),) * len(out_names)
    out_specs = (P("core"),) * len(out_names)
    fn = jax.jit(shard_map(_body, mesh=mesh, in_specs=in_specs,
                           out_specs=out_specs, check_rep=False))

    def _chain(n):
        def body_n(*args):
            ins, outbuf = args[:n_params], args[n_params]
            for _ in range(n):
                (outbuf,) = _body(*ins, outbuf)
            return (outbuf,)
        return jax.jit(shard_map(body_n, mesh=mesh, in_specs=in_specs,
                                 out_specs=out_specs, check_rep=False))

    runner = (fn, in_names, out_names, out_avals, mesh, _chain)
    _CACHE["runner"] = runner
    return runner


def _device_args(in_maps):
    """Assemble the jit arguments (host-side) for the runner."""
    fn, in_names, out_names, out_avals, mesh, _chain = _get_runner()
    args = []
    for n in in_names:
        if n == "hs":
            args.append(np.concatenate([m["hs"] for m in in_maps], axis=0))
        else:
            args.append(in_maps[0][n])
    for av in out_avals:
        args.append(np.zeros((NCORES * av.shape[0],) + av.shape[1:], av.dtype))
    return args


def kernel(**inputs) -> np.ndarray:
    fn, in_names, out_names, out_avals, mesh, _chain = _get_runner()
    in_maps = _prep_inputs(**inputs)
    args = _device_args(in_maps)
    outs = fn(*args)
    out = np.asarray(outs[0])  # (8*4096, 2048)
    return out.reshape(NCORES, B, TC, U, O).transpose(1, 0, 2, 3, 4).reshape(B, T, U, O)


if __name__ == "__main__":
    import time
    t0 = time.time()
    nc = build_nc(steps=int(sys.argv[1]) if len(sys.argv) > 1 else U)
    print(f"built ok in {time.time()-t0:.1f}s", flush=True)



# revision 48
# speedup vs baseline: 1.1384x; 1.1384x over previous
"""RNN-T decoder (embedding + 2-layer LSTM + joint network) on 8 Trainium2 cores.

v2 strategy (fp8 DoubleRow recurrence):
  - LSTM runs replicated on all 8 cores. The recurrent matmul streams W_hh as
    fp8(e4m3, x64) in DoubleRow perf mode (256 contraction rows per pass, 0.5
    cycles/output-row): 4x less PE time than the fp32r baseline. h is
    quantized to fp8 (x64) each step; the batched x-projection gx is
    quantized to fp8 (x32) and injected into PSUM via a DoubleRow matmul
    against a constant "double identity" whose entries are 128, so all PSUM
    contributions carry the same 4096x scale. Activations then apply
    tanh(ps/8192) (== sigmoid pre-halving) or tanh(ps/4096) for the g gate.
  - The elementwise c/h chain runs in a TRANSPOSED [128, 32] layout (gate
    values are transposed by the PE right after activation), which cuts
    DVE/ACT cost ~4x vs the [4, 512] layout and directly produces h^T tiles
    for the next step's matmul (fp8) and for xproj/joint (bf16).
  - Joint network is sharded over T (16 cols/core); zt/W_dec/W_out in bf16.

kernel(**inputs) takes the full unsharded inputs (as in reference.setup_inputs)
and returns the full (B, T, U, ODIM) float32 output.
"""
import sys
import numpy as np

sys.path.insert(0, "/opt/trn_rl_repo")

import concourse.bass as bass
import concourse.bacc as bacc
import concourse.mybir as mybir
import concourse.tile as tile
from contextlib import ExitStack

F32 = mybir.dt.float32
F32R = mybir.dt.float32r
BF16 = mybir.dt.bfloat16
F8 = mybir.dt.float8e4
I32 = mybir.dt.int32
AF = mybir.ActivationFunctionType
OP = mybir.AluOpType
DR = mybir.MatmulPerfMode.DoubleRow

B, T, U = 4, 128, 64
NCORES = 8
TC = T // NCORES          # 16 T-columns per core
E, D, G = 512, 1024, 4096  # embed, dunits, 4*dunits
J, O = 512, 2048           # joint dim, odim
UB = U * B                 # 256, u-major token index (col = 4u+b)
BT = B * TC                # 64 encoder rows per core
KD = D // 128              # 8 contraction chunks of hidden dim
DC = D // 256              # 4 double-chunks for DoubleRow

_CACHE = {}
JZT, JOSB, JPJ = 4, 4, 4
PBIG = 6


def _mm_r(nc, out, lhsT, rhs, **kw):
    """matmul with both operands viewed as float32r (full-rate fp32 storage)."""
    nc.tensor.matmul(out, lhsT=lhsT.bitcast(F32R), rhs=rhs.bitcast(F32R), **kw)


def _emit_xproj_q(nc, pools, rhs_of_ec, w_of_ec, nchunks, bih, gx8,
                  lhsT_bitcast=None):
    """gx8[2*nb+cp][:, i*256:(i+1)*256] = fp8(16 * ((W @ x^T) + bias)).

    rhs_of_ec: ec -> [128, 256] rhs tile (x^T chunk, K on partitions)
    w_of_ec:   ec -> [128, G] tile of W.T rows for that chunk (lhsT source)
    """
    pbig, bpool = pools["pbig"], pools["bias"]
    ones_r = pools["ones"]
    # one host-summed bias vector, loaded in a single DMA upfront (tiny DMAs
    # inside the loop would queue behind the weight streams and stall the
    # bias-inject matmuls)
    ba_all = bpool.tile([1, G], F32R, tag="ba_all", name="ba_all")
    nc.sync.dma_start(ba_all[:1, :], bih[None, :].bitcast(F32R))
    for gm in range(32):
        nb, c = gm // 4, gm % 4
        gs = slice(128 * c, 128 * c + 128)
        ps = pbig.tile([128, 256], F32, tag="pbig", name="pbig")
        for ec in range(nchunks):
            w = w_of_ec(ec)[:, 128 * gm: 128 * (gm + 1)]
            r = rhs_of_ec(ec)
            if lhsT_bitcast is None:
                _mm_r(nc, ps[:], lhsT=w, rhs=r,
                      start=(ec == 0), stop=False)
            else:
                nc.tensor.matmul(ps[:], lhsT=w, rhs=r,
                                 start=(ec == 0), stop=False)
        _mm_r(nc, ps[:], lhsT=ba_all[:1, 512 * nb + 128 * c:
                                     512 * nb + 128 * c + 128],
              rhs=ones_r[:1, :256], start=False, stop=True)
        # quantize to fp8 with x16 scale into the paired layout
        nc.vector.tensor_scalar(
            gx8[2 * nb + c // 2][:, (c % 2) * 256:(c % 2) * 256 + 256],
            ps[:], 32.0, None, OP.mult)


def _emit_lstm_fp8(nc, pools, whh8, gx8, hT16, cT, h8ab, steps, heat=0):
    """One LSTM layer in fp8 DoubleRow, `steps` sequential steps.

    whh8: 4 SBUF tiles [128, 2*G] fp8 (64x W_hh.T), dc-major double-chunks
    gx8: 16 SBUF tiles [128, 512] fp8 (16x (W_ih x + b)), paired layout
    hT16: [128, 32*(U+1)] bf16; col block u+1 <- h[u]^T, cols (i,dc,b)-major
    cT: [128, 32] f32 persistent (caller memsets)
    h8ab: two [128, 32] fp8 tiles, ping-pong (caller memsets h8ab[0])
    """
    pg8, ptT = pools["pgates"], pools["ptT"]
    tpool = pools["tsb"]
    ident8 = pools["ident8"]
    identb = pools["identb"]
    chain = pools["chain"]
    tc_t = pools["tc_t"]
    # processing order: i, g, f, o  (gate index 0,2,1,3) so the post-o tail
    # is only so/h8/h16
    ORDER = (0, 2, 1, 3)
    for u in range(steps):
        h8prev = h8ab[u % 2]
        h8cur = h8ab[(u + 1) % 2]
        lhs_h = h8prev[:].rearrange("p (i dc b) -> p dc i b", i=2, dc=DC, b=4)
        tT = ptT.tile([128, 128], BF16, tag="tT", name="tT")
        tsb = {}
        # --- all matmuls first (PE queue order) ---
        pgs = {}
        inj_first = pools.get("inj_first", False)
        for gt in ORDER:
            pg = pg8.tile([4, 1024], F32, tag="pg", name="pg")
            pgs[gt] = pg
            for nh in range(2):
                nb = 2 * gt + nh
                out = pg[:, 512 * nh: 512 * nh + 512]
                if inj_first:
                    for cp in range(2):
                        lg = gx8[2 * nb + cp][:].rearrange(
                            "p (i ub) -> p i ub", i=2)[:, :, 4 * u: 4 * u + 4]
                        nc.tensor.matmul(
                            pg[:, 512 * nh + 256 * cp: 512 * nh + 256 * cp + 256],
                            lhsT=lg, rhs=ident8[:].rearrange(
                                "p (i n) -> p i n", i=2),
                            start=(cp == 0), stop=False, perf_mode=DR)
                for dc in range(DC):
                    rhs = whh8[dc][:].rearrange("p (i n) -> p i n", i=2)[
                        :, :, 512 * nb: 512 * nb + 512]
                    nc.tensor.matmul(out, lhsT=lhs_h[:, dc], rhs=rhs,
                                     start=(dc == 0 and not inj_first),
                                     stop=(inj_first and dc == DC - 1),
                                     perf_mode=DR)
                if inj_first:
                    continue
                if "no_inj" in pools["ablate"]:
                    nc.tensor.matmul(
                        pg[:, 512 * nh: 512 * nh + 4], lhsT=lhs_h[:, 0],
                        rhs=whh8[0][:].rearrange("p (i n) -> p i n", i=2)[:, :, :4],
                        start=False, stop=True, perf_mode=DR)
                else:
                    for cp in range(2):
                        lg = gx8[2 * nb + cp][:].rearrange(
                            "p (i ub) -> p i ub", i=2)[:, :, 4 * u: 4 * u + 4]
                        nc.tensor.matmul(
                            pg[:, 512 * nh + 256 * cp: 512 * nh + 256 * cp + 256],
                            lhsT=lg, rhs=ident8[:].rearrange(
                                "p (i n) -> p i n", i=2),
                            start=False, stop=(cp == 1), perf_mode=DR)
        # --- activations + transposes + chain, in processing order ---
        dve_gates = pools.get("dve_gates") or ()
        split_first = pools.get("split_first", 0)
        for oi, gt in enumerate(ORDER):
            scale = (1.0 / 4096.0) if gt == 2 else (1.0 / 8192.0)
            t_sb = tpool.tile([4, 1024], BF16, tag="tsb", name="tsb")
            tsb[gt] = t_sb
            if "tiny_act" in pools["ablate"]:
                nc.scalar.activation(t_sb[:, :4], pgs[gt][:, :4], AF.Tanh,
                                     scale=scale)
                nc.gpsimd.memset(t_sb[:, 4:].bitcast(mybir.dt.uint16), 0)
            elif gt in dve_gates:
                # drain PSUM on the DVE (raw, pre-scaled); tanh after transpose
                nc.vector.tensor_scalar(t_sb[:], pgs[gt][:], scale, None,
                                        OP.mult)
            elif oi < split_first:
                # early gates: act per 512-half (each half is its own PSUM
                # accumulation group) so ACT starts half a block sooner
                for nh in range(2):
                    nc.scalar.activation(t_sb[:, 512 * nh:512 * nh + 512],
                                         pgs[gt][:, 512 * nh:512 * nh + 512],
                                         AF.Tanh, scale=scale)
            else:
                nc.scalar.activation(t_sb[:], pgs[gt][:], AF.Tanh, scale=scale)
            ntr = 1 if "no_tr" in pools["ablate"] else 8
            for k in range(ntr):
                ck = 16 * (k % 2) + 4 * (k // 2)
                nc.tensor.transpose(
                    tT[:, 32 * gt + ck: 32 * gt + ck + 4],
                    in_=t_sb[:, 128 * k: 128 * k + 128],
                    identity=identb[:4, :4])
            if ntr == 1:
                nc.gpsimd.memset(tT[:, 32 * gt + 4: 32 * gt + 32].bitcast(mybir.dt.uint16), 0)
            ts = tT[:, 32 * gt: 32 * gt + 32]
            if gt in dve_gates:
                ta = chain.tile([128, 32], BF16, tag=f"ta{gt}", name=f"ta{gt}")
                nc.scalar.activation(ta[:], ts, AF.Tanh)
                ts = ta[:]
            if gt == 0:      # i
                si = chain.tile([128, 32], F32, tag="si", name="si")
                pools["si"] = si
                nc.vector.tensor_scalar(si[:], ts, 0.5, 0.5, OP.mult, OP.add)
            elif gt == 2:    # g
                ig = chain.tile([128, 32], F32, tag="ig", name="ig")
                pools["ig"] = ig
                nc.vector.tensor_tensor(ig[:], in0=pools["si"][:], in1=ts,
                                        op=OP.mult)
            elif gt == 1:    # f
                sf = chain.tile([128, 32], F32, tag="sf", name="sf")
                nc.vector.tensor_scalar(sf[:], ts, 0.5, 0.5, OP.mult, OP.add)
                nc.vector.tensor_tensor(sf[:], in0=sf[:], in1=cT[:], op=OP.mult)
                nc.vector.tensor_tensor(cT[:], in0=sf[:], in1=pools["ig"][:],
                                        op=OP.add)
            else:            # o
                so = chain.tile([128, 32], F32, tag="so", name="so")
                pools["so"] = so
                nc.vector.tensor_scalar(so[:], ts, 32.0, 32.0, OP.mult, OP.add)
        # tanh(c') queues after the o activation so act_o isn't delayed
        nc.scalar.activation(tc_t[:], cT[:], AF.Tanh)
        nc.vector.tensor_tensor(h8cur[:], in0=pools["so"][:], in1=tc_t[:],
                                op=OP.mult)
        nc.vector.tensor_scalar(
            hT16[:, 32 * (u + 1): 32 * (u + 2)], h8cur[:],
            1.0 / 64.0, None, OP.mult)
        # optional PE heater: keep the tensor engine clocked during the tail
        for _ in range(heat):
            hp = pools["pheat"].tile([4, 512], F32, tag="ph", name="ph")
            nc.tensor.matmul(hp[:], lhsT=lhs_h[:, 0],
                             rhs=whh8[0][:].rearrange(
                                 "p (i n) -> p i n", i=2)[:, :, :512],
                             start=True, stop=True, perf_mode=DR)


def build_nc(steps=U, layers=2, joint=True, heat=0, pg_bufs=3,
             dve_gates=(), prefetch=1, ablate=(), inj_first=1, split_first=2):
    nc = bacc.Bacc("TRN2", target_bir_lowering=False, debug=False)

    hs = nc.dram_tensor("hs", [BT, E], F32, kind="ExternalInput")
    ys_idx = nc.dram_tensor("ys_idx", [UB], I32, kind="ExternalInput")
    embed = nc.dram_tensor("embed", [O, E], F32, kind="ExternalInput")
    wih0T = nc.dram_tensor("wih0T", [E, G], BF16, kind="ExternalInput")
    wih1T16 = nc.dram_tensor("wih1T16", [D, G], BF16, kind="ExternalInput")
    whh8_0 = nc.dram_tensor("whh8_0", [4 * 128, 2 * G], F8, kind="ExternalInput")
    whh8_1 = nc.dram_tensor("whh8_1", [4 * 128, 2 * G], F8, kind="ExternalInput")
    bih0 = nc.dram_tensor("bih0", [G], F32, kind="ExternalInput")
    bih1 = nc.dram_tensor("bih1", [G], F32, kind="ExternalInput")
    wencT = nc.dram_tensor("wencT", [E, J], F32R, kind="ExternalInput")
    benc = nc.dram_tensor("benc", [J], F32R, kind="ExternalInput")
    wdecT16 = nc.dram_tensor("wdecT16", [D, J], BF16, kind="ExternalInput")
    woutT16 = nc.dram_tensor("woutT16", [J, O], BF16, kind="ExternalInput")
    bout_bc = nc.dram_tensor("bout_bc", [128, O], BF16, kind="ExternalInput")
    ones_d = nc.dram_tensor("ones_d", [256], F32R, kind="ExternalInput")
    ident_f = nc.dram_tensor("ident_f", [128, 128], F32, kind="ExternalInput")
    ident_b = nc.dram_tensor("ident_b", [128, 128], BF16, kind="ExternalInput")
    ident_8 = nc.dram_tensor("ident_8", [128, 512], F8, kind="ExternalInput")
    out = nc.dram_tensor("out", [BT * U, O], F32, kind="ExternalOutput")

    with tile.TileContext(nc) as tc, ExitStack() as es:
        cpool = es.enter_context(tc.tile_pool(name="const", bufs=1))
        ppool = es.enter_context(tc.tile_pool(name="persist", bufs=1))

        ident = cpool.tile([128, 128], F32, tag="ident", name="ident")
        nc.sync.dma_start(ident[:], ident_f[:])
        ones_r = cpool.tile([1, 256], F32R, tag="ones", name="ones")
        nc.sync.dma_start(ones_r[:1, :], ones_d[None, :])
        identb = cpool.tile([128, 128], BF16, tag="identb", name="identb")
        nc.sync.dma_start(identb[:], ident_b[:])
        ident8 = cpool.tile([128, 512], F8, tag="ident8", name="ident8")
        nc.sync.dma_start(ident8[:], ident_8[:])

        gx8 = [ppool.tile([128, 512], F8, tag=f"gx8_{g}", name=f"gx8_{g}")
               for g in range(16)]
        hT16_0 = ppool.tile([128, 32 * (U + 1)], BF16, tag="hT16_0", name="hT16_0")
        hT16_1 = ppool.tile([128, 32 * (U + 1)], BF16, tag="hT16_1", name="hT16_1")
        cT = ppool.tile([128, 32], F32, tag="cT", name="cT")
        h8a = ppool.tile([128, 32], F8, tag="h8a", name="h8a")
        h8b = ppool.tile([128, 32], F8, tag="h8b", name="h8b")
        tc_t = ppool.tile([128, 32], BF16, tag="tc_t", name="tc_t")
        whh0 = [ppool.tile([128, 2 * G], F8, tag=f"whh0_{d}", name=f"whh0_{d}")
                for d in range(DC)]
        whh1 = [ppool.tile([128, 2 * G], F8, tag=f"whh1_{d}", name=f"whh1_{d}")
                for d in range(DC)]

        pools = {"ones": ones_r, "identb": identb, "ident8": ident8,
                 "tc_t": tc_t, "dve_gates": tuple(dve_gates),
                 "ablate": tuple(ablate), "inj_first": bool(inj_first),
                 "split_first": split_first}

        wenc = [ppool.tile([128, J], F32R, tag=f"wenc{ec}", name=f"wenc{ec}")
                for ec in range(4)]
        benc_sb = ppool.tile([1, J], F32R, tag="benc", name="benc")
        hs_sb = ppool.tile([BT, E], F32, tag="hs_sb", name="hs_sb")
        hsT = [ppool.tile([128, BT], F32R, tag=f"hsT{ec}", name=f"hsT{ec}")
               for ec in range(4)]
        encp = [ppool.tile([128, BT], F32, tag=f"encp{jt}", name=f"encp{jt}")
                for jt in range(4)]

        # ---- Phase 1+2: embedding gather -> eys^T, layer-0 x-projection ----
        with tc.tile_pool(name="ph2", bufs=1) as p2, \
             tc.tile_pool(name="bias2", bufs=2) as bpool2, \
             tc.tile_pool(name="pbig", bufs=PBIG, space="PSUM") as pbig, \
             tc.tile_pool(name="pT", bufs=2, space="PSUM") as pT:
            pools["pbig"] = pbig
            pools["bias"] = bpool2
            eysT = [p2.tile([128, 256], BF16, tag=f"eysT{ec}", name=f"eysT{ec}")
                    for ec in range(4)]
            idxs = []
            for t in range(2):
                idx = p2.tile([128, 1], I32, tag=f"idx{t}", name=f"idx{t}")
                nc.sync.dma_start(idx[:, :1], ys_idx[128 * t:128 * (t + 1), None])
                idxs.append(idx)
            wih0 = [p2.tile([128, G], BF16, tag=f"wih0_{ec}", name=f"wih0_{ec}")
                    for ec in range(4)]
            for ec in range(4):
                nc.sync.dma_start(wih0[ec][:], wih0T[128 * ec:128 * (ec + 1), :])
            # recurrent weights (both layers) early: overlaps with xproj
            for d in range(DC):
                nc.sync.dma_start(whh0[d][:], whh8_0[128 * d:128 * (d + 1), :])
            for t in range(2):
                idx = idxs[t]
                ey = p2.tile([128, E], F32, tag=f"ey{t}", name=f"ey{t}")
                nc.gpsimd.indirect_dma_start(
                    out=ey[:], out_offset=None, in_=embed[:],
                    in_offset=bass.IndirectOffsetOnAxis(ap=idx[:, :1], axis=0))
                for ec in range(4):
                    tp = pT.tile([128, 128], F32, tag="pT", name="pT")
                    nc.tensor.transpose(tp[:], in_=ey[:, 128 * ec:128 * (ec + 1)],
                                        identity=ident[:])
                    nc.vector.tensor_copy(eysT[ec][:, 128 * t:128 * (t + 1)], tp[:])
            _emit_xproj_q(nc, pools, lambda ec: eysT[ec][:],
                          lambda ec: wih0[ec], 4, bih0, gx8,
                          lhsT_bitcast=True)

            for d in range(DC):
                nc.sync.dma_start(whh1[d][:], whh8_1[128 * d:128 * (d + 1), :])
            # encoder-side joint work: hs^T and enc_p^T (PE has slack here)
            for ec in range(4):
                nc.sync.dma_start(wenc[ec][:], wencT[128 * ec:128 * (ec + 1), :])
            nc.sync.dma_start(hs_sb[:], hs[:])
            nc.sync.dma_start(benc_sb[:1, :], benc[None, :])
            for ec in range(4):
                tp = pT.tile([128, 128], F32, tag="pT", name="pT")
                nc.tensor.transpose(tp[:, :BT],
                                    in_=hs_sb[:, 128 * ec:128 * (ec + 1)],
                                    identity=ident[:BT, :BT])
                nc.vector.tensor_copy(hsT[ec][:], tp[:, :BT])
            for jt in range(4):
                tpp = pT.tile([128, 128], F32, tag="pT", name="pT")
                pse = tpp[:, :BT]
                for ec in range(4):
                    _mm_r(nc, pse, lhsT=wenc[ec][:, 128 * jt:128 * (jt + 1)],
                          rhs=hsT[ec][:], start=(ec == 0), stop=False)
                _mm_r(nc, pse, lhsT=benc_sb[:1, 128 * jt:128 * (jt + 1)],
                      rhs=ones_r[:1, :BT], start=False, stop=True)
                nc.vector.tensor_copy(encp[jt][:], pse)

        # ---- Phase 3: layer-0 recurrence ----
        wih1p = es.enter_context(tc.tile_pool(name="wih1p", bufs=1))
        wih1 = [wih1p.tile([128, G], BF16, tag=f"wih1_{k}", name=f"wih1_{k}")
                for k in range(KD)]
        if prefetch:
            for k in range(KD):
                nc.sync.dma_start(wih1[k][:], wih1T16[128 * k:128 * (k + 1), :])
        if steps < U:  # debug builds: phases 4/6 read all U step blocks
            nc.gpsimd.memset(hT16_0[:], 0.0)
            nc.gpsimd.memset(hT16_1[:], 0.0)
        nc.gpsimd.memset(cT[:], 0.0)
        nc.gpsimd.memset(h8a[:].bitcast(mybir.dt.uint8), 0)
        with tc.tile_pool(name="pgA", bufs=pg_bufs, space="PSUM") as pgates, \
             tc.tile_pool(name="ptTA", bufs=2, space="PSUM") as ptT, \
             tc.tile_pool(name="pheatA", bufs=1, space="PSUM") as pheat, \
             tc.tile_pool(name="tsbA", bufs=3) as tsb, \
             tc.tile_pool(name="chainA", bufs=2) as chain:
            pools.update(pgates=pgates, ptT=ptT, tsb=tsb, chain=chain,
                         pheat=pheat)
            _emit_lstm_fp8(nc, pools, whh0, gx8, hT16_0, cT, (h8a, h8b),
                           steps, heat=heat)

        # ---- Phase 4: layer-1 x-projection (streams W_ih1 bf16) ----
        with tc.tile_pool(name="bias4", bufs=2) as bpool4, \
             tc.tile_pool(name="pbig2", bufs=4, space="PSUM") as pbig2:
            pools["pbig"] = pbig2
            pools["bias"] = bpool4
            if not prefetch:
                for k in range(KD):
                    nc.sync.dma_start(wih1[k][:],
                                      wih1T16[128 * k:128 * (k + 1), :])
            rhs_of = lambda ec: hT16_0[:].rearrange(
                "p (u c b) -> p c u b", u=U + 1, c=8, b=4)[
                :, 4 * (ec % 2) + ec // 2, 1:U + 1]
            _emit_xproj_q(nc, pools, rhs_of, lambda ec: wih1[ec], KD,
                          bih1, gx8, lhsT_bitcast=True)

        # ---- Phase 5: layer-1 recurrence ----
        if joint:
            jp = es.enter_context(tc.tile_pool(name="joint", bufs=1))
            wdec = [jp.tile([128, J], BF16, tag=f"wdec{k}", name=f"wdec{k}")
                    for k in range(KD)]
            wout = [jp.tile([128, O], BF16, tag=f"wout{jt}", name=f"wout{jt}")
                    for jt in range(4)]
            bout_sb = jp.tile([128, O], BF16, tag="bout", name="bout")
            if prefetch:
                for k in range(KD):
                    nc.sync.dma_start(wdec[k][:],
                                      wdecT16[128 * k:128 * (k + 1), :])
                nc.sync.dma_start(bout_sb[:], bout_bc[:])
                for jt in range(4):
                    nc.sync.dma_start(wout[jt][:],
                                      woutT16[128 * jt:128 * (jt + 1), :])
        nc.gpsimd.memset(cT[:], 0.0)
        nc.gpsimd.memset(h8a[:].bitcast(mybir.dt.uint8), 0)
        if layers >= 2:
            with tc.tile_pool(name="pgB", bufs=pg_bufs, space="PSUM") as pgates2, \
                 tc.tile_pool(name="ptTB", bufs=2, space="PSUM") as ptT2, \
                 tc.tile_pool(name="pheatB", bufs=1, space="PSUM") as pheat2, \
                 tc.tile_pool(name="tsbB", bufs=3) as tsb2, \
                 tc.tile_pool(name="chainB", bufs=2) as chain2:
                pools.update(pgates=pgates2, ptT=ptT2, tsb=tsb2, chain=chain2,
                             pheat=pheat2)
                _emit_lstm_fp8(nc, pools, whh1, gx8, hT16_1, cT, (h8a, h8b),
                               steps, heat=heat)
        else:
            nc.vector.tensor_copy(hT16_1[:, 32:32 * (steps + 1)],
                                  hT16_0[:, 32:32 * (steps + 1)])

        # ---- Phase 6: joint network on this core's T-slice ----
        if joint:
            with tc.tile_pool(name="jointl", bufs=1) as jp2, \
                 tc.tile_pool(name="zt", bufs=JZT) as ztp, \
                 tc.tile_pool(name="osb", bufs=JOSB) as osbp, \
                 tc.tile_pool(name="pj", bufs=JPJ, space="PSUM") as pj:
                decp = [jp2.tile([128, 256], F32, tag=f"decp{jt}", name=f"decp{jt}")
                        for jt in range(4)]
                if not prefetch:
                    for k in range(KD):
                        nc.sync.dma_start(wdec[k][:],
                                          wdecT16[128 * k:128 * (k + 1), :])
                    nc.sync.dma_start(bout_sb[:], bout_bc[:])
                    for jt in range(4):
                        nc.sync.dma_start(wout[jt][:],
                                          woutT16[128 * jt:128 * (jt + 1), :])

                # dec_p^T[jt], columns reordered (b, u)
                for jt in range(4):
                    ps = pj.tile([128, 256], F32, tag="pj", name="pj")
                    for k in range(KD):
                        rhs = hT16_1[:].rearrange(
                            "p (u c b) -> p c b u", u=U + 1, c=8, b=4)[
                            :, 4 * (k % 2) + k // 2, :, 1:U + 1]
                        nc.tensor.matmul(ps[:], lhsT=wdec[k][:, 128 * jt:128 * (jt + 1)],
                                         rhs=rhs, start=(k == 0), stop=(k == KD - 1))
                    nc.vector.tensor_copy(decp[jt][:], ps[:])
                # z^T tiles + output matmul, one M-tile (=2 encoder rows) at a time
                for m in range(BT * U // 128):
                    zt = [ztp.tile([128, 128], BF16, tag=f"zt{jt}", name=f"zt{jt}")
                          for jt in range(4)]
                    for jt in range(4):
                        for half in range(2):
                            bt = 2 * m + half
                            b = bt // TC
                            nc.scalar.activation(
                                zt[jt][:, half * 64:(half + 1) * 64],
                                decp[jt][:, b * 64:(b + 1) * 64],
                                AF.Tanh, bias=encp[jt][:, bt:bt + 1])
                    for ob in range(4):
                        obs = slice(ob * 512, (ob + 1) * 512)
                        ps = pj.tile([128, 512], F32, tag="pj", name="pj")
                        for jt in range(4):
                            nc.tensor.matmul(ps[:], lhsT=zt[jt][:],
                                             rhs=wout[jt][:, obs],
                                             start=(jt == 0), stop=(jt == 3))
                        o_sb = osbp.tile([128, 512], F32, tag="osb", name="osb")
                        nc.vector.tensor_tensor(o_sb[:], in0=ps[:],
                                                in1=bout_sb[:, obs], op=OP.add)
                        nc.sync.dma_start(out[128 * m:128 * (m + 1), obs], o_sb[:])
        else:
            zsrc = ppool.tile([128, 512], F32, tag="zsrc", name="zsrc")
            nc.gpsimd.memset(zsrc[:], 0.0)
            for m0 in range(BT * U // 128):
                for ob in range(4):
                    nc.sync.dma_start(out[128 * m0:128 * (m0 + 1),
                                          ob * 512:(ob + 1) * 512], zsrc[:])

    nc.compile()
    return nc


def _prep_inputs(hs_pad, ys_in_pad, embed, W_ih0, W_hh0, b_ih0, b_hh0,
                 W_ih1, W_hh1, b_ih1, b_hh1, W_enc, b_enc, W_dec, W_out, b_out):
    import ml_dtypes
    E4 = ml_dtypes.float8_e4m3
    BF = ml_dtypes.bfloat16
    f = np.float32
    tr = lambda a: np.ascontiguousarray(np.asarray(a).T, dtype=f)

    def whh8(W):
        WT64 = (np.asarray(W, f).T * 64.0).astype(E4)          # [D, G]
        return np.ascontiguousarray(
            WT64.reshape(DC, 2, 128, G).transpose(0, 2, 1, 3).reshape(4 * 128, 2 * G))

    id8 = np.zeros((128, 512), E4)
    for p in range(128):
        id8[p, p] = 128.0
        id8[p, 256 + 128 + p] = 128.0

    common = {
        "ys_idx": np.ascontiguousarray(np.asarray(ys_in_pad).T.reshape(-1),
                                       dtype=np.int32),
        "embed": np.ascontiguousarray(embed, dtype=f),
        "wih0T": np.ascontiguousarray(np.asarray(W_ih0, f).T.astype(BF)),
        "wih1T16": np.ascontiguousarray(np.asarray(W_ih1, f).T.astype(BF)),
        "whh8_0": whh8(W_hh0), "whh8_1": whh8(W_hh1),
        "bih0": np.asarray(b_ih0, f) + np.asarray(b_hh0, f),
        "bih1": np.asarray(b_ih1, f) + np.asarray(b_hh1, f),
        "wencT": tr(W_enc),
        "wdecT16": np.ascontiguousarray(np.asarray(W_dec, f).T.astype(BF)),
        "woutT16": np.ascontiguousarray(np.asarray(W_out, f).T.astype(BF)),
        "benc": np.asarray(b_enc, f),
        "bout_bc": np.ascontiguousarray(
            np.broadcast_to(np.asarray(b_out, f).astype(BF)[None, :], (128, O))),
        "ones_d": np.ones(256, f),
        "ident_f": np.eye(128, dtype=f),
        "ident_b": np.eye(128).astype(BF),
        "ident_8": id8,
    }
    hs_np = np.asarray(hs_pad, f)
    in_maps = []
    for c in range(NCORES):
        m = dict(common)
        m["hs"] = np.ascontiguousarray(
            hs_np[:, c * TC:(c + 1) * TC, :].reshape(BT, E))
        in_maps.append(m)
    return in_maps



def _get_runner():
    """Build (once) a reusable jitted SPMD callable.

    Weights are replicated across the 8 cores (in_specs=P()); only hs and the
    output are sharded over the leading axis. This avoids the 8x concat +
    retrace of run_bass_kernel_spmd on every call.
    """
    if "runner" in _CACHE:
        return _CACHE["runner"]
    import jax
    from jax.sharding import Mesh, PartitionSpec as P
    from jax.experimental.shard_map import shard_map
    from concourse import bass2jax
    import concourse.mybir as mybir_

    nc = _CACHE.get("nc")
    if nc is None:
        nc = _CACHE["nc"] = build_nc()
    bass2jax.install_neuronx_cc_hook()

    pname = nc.partition_id_tensor.name if nc.partition_id_tensor else None
    in_names, out_names, out_avals = [], [], []
    for alloc in nc.m.functions[0].allocations:
        if not isinstance(alloc, mybir_.MemoryLocationSet):
            continue
        name = alloc.memorylocations[0].name
        if alloc.kind == "ExternalInput":
            if name != pname:
                in_names.append(name)
        elif alloc.kind == "ExternalOutput":
            out_names.append(name)
            shape = tuple(alloc.tensor_shape)
            out_avals.append(jax.core.ShapedArray(shape, mybir_.dt.np(alloc.dtype)))
    n_params = len(in_names)
    all_names = in_names + out_names
    if pname is not None:
        all_names = all_names + [pname]

    def _body(*args):
        operands = list(args)
        if pname is not None:
            operands.append(bass2jax.partition_id_tensor())
        outs = bass2jax._bass_exec_p.bind(
            *operands,
            out_avals=tuple(out_avals),
            in_names=tuple(all_names),
            out_names=tuple(out_names),
            lowering_input_output_aliases=(),
            sim_require_finite=True,
            sim_require_nnan=True,
            nc=nc,
        )
        return tuple(outs)

    devices = jax.devices()[:NCORES]
    mesh = Mesh(np.asarray(devices), ("core",))
    in_specs = tuple(P("core") if n == "hs" else P() for n in in_names)
    in_specs = in_specs + (P("core"),) * len(out_names)
    out_specs = (P("core"),) * len(out_names)
    fn = jax.jit(shard_map(_body, mesh=mesh, in_specs=in_specs,
                           out_specs=out_specs, check_rep=False))

    def _chain(n):
        def body_n(*args):
            ins, outbuf = args[:n_params], args[n_params]
            for _ in range(n):
                (outbuf,) = _body(*ins, outbuf)
            return (outbuf,)
        return jax.jit(shard_map(body_n, mesh=mesh, in_specs=in_specs,
                                 out_specs=out_specs, check_rep=False))

    runner = (fn, in_names, out_names, out_avals, mesh, _chain)
    _CACHE["runner"] = runner
    return runner


def _device_args(in_maps):
    """Assemble the jit arguments (host-side) for the runner."""
    fn, in_names, out_names, out_avals, mesh, _chain = _get_runner()
    args = []
    for n in in_names:
        if n == "hs":
            args.append(np.concatenate([m["hs"] for m in in_maps], axis=0))
        else:
            args.append(in_maps[0][n])
    for av in out_avals:
        args.append(np.zeros((NCORES * av.shape[0],) + av.shape[1:], av.dtype))
    return args


def kernel(**inputs) -> np.ndarray:
    fn, in_names, out_names, out_avals, mesh, _chain = _get_runner()
    in_maps = _prep_inputs(**inputs)
    args = _device_args(in_maps)
    outs = fn(*args)
    out = np.asarray(outs[0])  # (8*4096, 2048)
    return out.reshape(NCORES, B, TC, U, O).transpose(1, 0, 2, 3, 4).reshape(B, T, U, O)


if __name__ == "__main__":
    import time
    t0 = time.time()
    nc = build_nc(steps=int(sys.argv[1]) if len(sys.argv) > 1 else U)
    print(f"built ok in {time.time()-t0:.1f}s", flush=True)

